# revision 24
# baseline (speedup 1.0000x reference)
"""CMMD loss kernel for Trainium2 (Bass/Tile), 8-core SPMD, collective-free.

Math (reference semantics):
  X = concat(source, target)            [N, D], N=4096, D=2048
  L2[i,j] = ||X_i - X_j||^2
  bw  = sum(L2) / (N^2 - N) / 4         (colsum term ~1e-4 relative, dropped)
  K   = sum_{l=0..4} exp(-L2 / (bw * 2^l))
  loss = (1/bs^2) * sum_{ij} (V_i . V_j) K_ij,  V_i = sign_i * onehot(label_i)

Distribution: full replication of X^T in fp8 (e4m3) on every core; core c
computes the 512-row panel rows [512c, 512c+512) and a scalar partial; the
host sums 8 partials.  No collectives -> no cross-core rendezvous.

Per core:
 - SBUF holds full X^T as 8 fp8 tiles [128, 2, 4096] (DoubleRow k-pairs) plus
   the core's own column block [128, 2, 512] (separate per-core input).
 - Row norms ||x_j||^2 for all j: ACT/DVE squares of the fp8 tiles (exact in
   bf16) + ones-matmul partition reduction into PSUM [1,512] chunks packed 4
   per bank at partition offsets {0,32,64,96}.
 - nh = -0.5*||x||^2 split bf16 hi+lo; folded into the Gram as 4 augmented
   contraction rows (ones x nh_j + nh_i x ones), so PSUM holds
   P = x_i.x_j - 0.5||x_i||^2 - 0.5||x_j||^2 = -L2/2 and the exp needs only a
   per-partition scale 2/sigma_l.
 - Gram panel: fp8 DoubleRow matmuls, pass structure (jt-group of <=3, i) so
   one weight load feeds 3 matmuls; PSUM 6 gram banks ping-pong + 2 R banks.
 - E4 = exp(sc4*P) (ACT), then 4 bf16 squarings + 4 adds (DVE) build
   K = sum_l E_l; one matmul V_blk^T @ K accumulates R[c, j] per column tile
   (R tiles packed 4-per-bank at partition offsets 32*j).
 - loss_cols via fused DVE tensor_tensor_reduce against V^T replicated at the
   same partition offsets; final ones-matmul contraction -> scalar partial.
"""

import os
from dataclasses import dataclass

import numpy as np
import ml_dtypes

import concourse.bass as bass
import concourse.bacc as bacc
import concourse.mybir as mybir
import concourse.tile as tile

F32 = mybir.dt.float32
BF16 = mybir.dt.bfloat16
F8E4 = mybir.dt.float8e4
AX = mybir.AxisListType
ALU = mybir.AluOpType
ACTF = mybir.ActivationFunctionType
DR = mybir.MatmulPerfMode.DoubleRow


@dataclass(frozen=True)
class Cfg:
    n: int = 4096          # total rows (source + target)
    d: int = 2048          # features
    cores: int = 8
    ncls: int = 8          # one-hot classes, padded 7 -> 8
    nl: int = 5            # kernel_num

    @property
    def rpc(self):   # rows per core
        return self.n // self.cores

    @property
    def ni(self):    # 128-row blocks per core panel
        return self.rpc // 128

    @property
    def nkk(self):   # DoubleRow contraction pairs (2x128 each)
        return self.d // 256

    @property
    def nj(self):    # 512-wide column tiles
        return self.n // 512


CFG = Cfg()
GROUPS = [(0, 1, 2), (3, 4, 5), (6, 7)]


def _build(cfg: Cfg):
    nc = bacc.Bacc(
        "TRN2", target_bir_lowering=False, debug=False, num_devices=1
    )
    N, NI, NKK, NJ, NC, NL = cfg.n, cfg.ni, cfg.nkk, cfg.nj, cfg.ncls, cfg.nl

    xt8 = nc.dram_tensor("xt8", [NKK, 128, 2 * N], F8E4, kind="ExternalInput").ap()
    xto8 = nc.dram_tensor("xto8", [NKK, 128, 2 * cfg.rpc], F8E4, kind="ExternalInput").ap()
    vown = nc.dram_tensor("vown", [128, NI * NC], BF16, kind="ExternalInput").ap()
    vt4 = nc.dram_tensor("vt4", [128, N], BF16, kind="ExternalInput").ap()
    cones = nc.dram_tensor("cones", [128, 1], BF16, kind="ExternalInput").ap()
    conesf = nc.dram_tensor("conesf", [128, 1], F32, kind="ExternalInput").ap()
    crowf = nc.dram_tensor("crowf", [1, 128], F32, kind="ExternalInput").ap()
    cst = nc.dram_tensor("cst", [1, 16], F32, kind="ExternalInput").ap()
    conesN = nc.dram_tensor("conesN", [1, 4096], BF16, kind="ExternalInput").ap()
    partial = nc.dram_tensor("partial", [1, 1], F32, kind="ExternalOutput").ap()

    with tile.TileContext(nc) as tc:
        with (
            tc.tile_pool(name="dram", bufs=1, space="DRAM") as dram,
            tc.tile_pool(name="pers", bufs=1) as pers,
        ):
            laux_dram = dram.tile([1, N], BF16)
            xt = [pers.tile([128, 2, N], F8E4, name=f"xt{k}") for k in range(NKK)]
            xto = [pers.tile([128, 2, cfg.rpc], F8E4, name=f"xto{k}") for k in range(NKK)]
            vown_sb = pers.tile([128, NI, NC], BF16)
            vt4_sb = pers.tile([128, N], BF16)
            ones_col = pers.tile([128, 1], BF16)
            onesf_col = pers.tile([128, 1], F32)
            onesf_row = pers.tile([1, 128], F32)
            cst_sb = pers.tile([1, 16], F32)
            sc = pers.tile([128, 8], F32)
            laux = pers.tile([2, N], BF16)          # rhs aug rows: nhh | 1
            lext = [pers.tile([2, 128], BF16, name=f"lext{i}") for i in range(NI)]
            loss_cols = pers.tile([128, NJ], F32)
            lred = pers.tile([128, 1], F32)
            out_sb = pers.tile([1, 1], F32)

            # constants + small inputs first, then own block, then full X^T
            nc.sync.dma_start(ones_col[:], cones)
            nc.sync.dma_start(onesf_col[:], conesf)
            nc.sync.dma_start(onesf_row[:], crowf)
            nc.sync.dma_start(cst_sb[:], cst)
            nc.sync.dma_start(vown_sb[:], vown.rearrange("p (i c) -> p i c", c=NC))
            nc.sync.dma_start(vt4_sb[:], vt4)
            for k in range(NKK):
                nc.sync.dma_start(
                    xto[k][:], xto8[k].rearrange("p (t c) -> p t c", t=2)
                )
            for k in range(NKK):
                nc.sync.dma_start(
                    xt[k][:], xt8[k].rearrange("p (t c) -> p t c", t=2)
                )

            nc.sync.dma_start(laux[1:2, :], conesN)
            nc.vector.memset(loss_cols[:], 0.0)
            for i in range(NI):
                nc.vector.memset(lext[i][0:1, :], 1.0)

            with (
                tc.tile_pool(name="pre", bufs=1) as pre,
                tc.tile_pool(name="prep", bufs=1, space="PSUM") as prep,
            ):
                # norm chunk banks, 3 chunks per bank at partition offsets
                # {0, 32, 64}: A holds jt 0-2, B holds 3-5, C holds 6-7 + own@64
                nrm = [
                    prep.tile([128, 512], F32, tag=f"n{b}", name=f"nrm{b}")
                    for b in range(3)
                ]
                CHUNKS = [(0, [0, 1, 2]), (1, [3, 4, 5]), (2, [6, 7])]

                def chunk_ap(jt):
                    b, off = jt // 3, 32 * (jt % 3)
                    return nrm[b][off : off + 1, :]

                own_ap = nrm[2][64:65, :]

                for k in range(NKK):
                    sqo = pre.tile([128, 2, cfg.rpc], BF16, tag="sqo", bufs=2)
                    nc.scalar.activation(sqo[:], xto[k][:], ACTF.Square)
                    for t in range(2):
                        nc.tensor.matmul(
                            own_ap,
                            lhsT=ones_col[:],
                            rhs=sqo[:, t, :],
                            start=(k == 0 and t == 0),
                            stop=(k == NKK - 1 and t == 1),
                        )
                    sqa = pre.tile([128, 2, N], BF16, tag="sqa", bufs=2)
                    if k % 2 == 0:
                        nc.scalar.activation(sqa[:], xt[k][:], ACTF.Square)
                    else:
                        nc.vector.tensor_tensor(
                            sqa[:], xt[k][:], xt[k][:], op=ALU.mult
                        )
                    for t in range(2):
                        for jt in range(NJ):
                            nc.tensor.matmul(
                                chunk_ap(jt),
                                lhsT=ones_col[:],
                                rhs=sqa[:, t, 512 * jt : 512 * (jt + 1)],
                                start=(k == 0 and t == 0),
                                stop=(k == NKK - 1 and t == 1),
                            )

                # nh = -0.5*norm in bf16 (hi only; lo residual verified
                # negligible), assembled into laux row 0 via DRAM bounce
                hstack = pre.tile([8, 512], BF16, tag="hstack", bufs=1)
                for jt in range(NJ):
                    base = 32 * (jt % 3)
                    h_t = pre.tile([128, 512], BF16, tag="th", bufs=3, name=f"th{jt}")
                    hv = h_t[base : base + 1, :]
                    nc.vector.tensor_scalar_mul(hv, chunk_ap(jt), -0.5)
                    eng = nc.sync if jt % 2 == 0 else nc.scalar
                    eng.dma_start(laux_dram[0:1, 512 * jt : 512 * (jt + 1)], hv)
                    eng2 = nc.scalar if jt % 2 == 0 else nc.sync
                    eng2.dma_start(hstack[jt : jt + 1, :], hv)
                nc.sync.dma_start(laux[0:1, :], laux_dram[:])

                oh_t = pre.tile([128, 512], BF16, tag="oh", bufs=1)
                oh = oh_t[64:65, :]
                nc.vector.tensor_scalar_mul(oh, own_ap, -0.5)
                for i in range(NI):
                    nc.scalar.dma_start(
                        lext[i][1:2, :], oh[:, 128 * i : 128 * (i + 1)]
                    )

                # bandwidth: s1 = -2 * sum(nh chunks) via the SBUF stack
                # (skips the laux DRAM-roundtrip latency on the sc path)
                red8 = pre.tile([8, 1], F32, tag="red8", bufs=1)
                s1 = pre.tile([1, 1], F32, tag="sc1", bufs=8)
                inv = pre.tile([1, 1], F32, tag="sc1", bufs=8)
                nc.vector.tensor_reduce(red8[:], hstack[:], axis=AX.X, op=ALU.add)
                psum_s1 = prep.tile([1, 1], F32, tag="s1b")
                nc.tensor.matmul(
                    psum_s1[:], lhsT=red8[:], rhs=onesf_col[0:8, :],
                    start=True, stop=True,
                )
                nc.vector.tensor_scalar_mul(s1[:], psum_s1[:], -2.0)
                nc.vector.reciprocal(inv[:], s1[:])
                sc_row = pre.tile([1, 16], F32, tag="scr", bufs=1)
                nc.vector.tensor_scalar_mul(sc_row[:], cst_sb[:], inv[:])
                psum_b = prep.tile([128, 16], F32, tag="scb")
                nc.tensor.matmul(
                    psum_b[:], lhsT=onesf_row[:], rhs=sc_row[:], start=True, stop=True
                )
                nc.vector.tensor_copy(sc[:], psum_b[:, 0:8])

            with (
                tc.tile_pool(name="work", bufs=1) as work,
                tc.tile_pool(name="mpsum", bufs=1, space="PSUM") as mpsum,
            ):
                passes = []
                for grp in GROUPS:
                    for i in range(NI):
                        passes.append((grp, i))

                racc_of_group = {}
                prev = None  # (grp, i, Ktiles)
                for grp, i in passes:
                    if i == 0:
                        racc_of_group[grp] = [None] * len(grp)

                    gs = [
                        mpsum.tile(
                            [128, 512], F32, tag="g", bufs=6,
                            name=f"g_{grp[0]}_{i}_{j}",
                        )
                        for j in grp
                    ]
                    for k in range(NKK):
                        for j_idx, jt in enumerate(grp):
                            nc.tensor.matmul(
                                gs[j_idx],
                                lhsT=xto[k][:, :, 128 * i : 128 * (i + 1)],
                                rhs=xt[k][:, :, 512 * jt : 512 * (jt + 1)],
                                start=(k == 0),
                                stop=False,
                                perf_mode=DR,
                            )
                    for j_idx, jt in enumerate(grp):
                        nc.tensor.matmul(
                            gs[j_idx],
                            lhsT=lext[i][:],
                            rhs=laux[:, 512 * jt : 512 * (jt + 1)],
                            start=False,
                            stop=True,
                        )

                    # previous pass's V^T @ K reduce (its chain is done by now)
                    if prev is not None:
                        pgrp, pi, pK = prev
                        _emit_reduce(
                            nc, work, mpsum, pgrp, pi, pK, vown_sb,
                            racc_of_group[pgrp],
                        )
                        if pi == NI - 1:
                            _emit_group_tail(
                                nc, work, pgrp, racc_of_group[pgrp], vt4_sb,
                                loss_cols,
                            )

                    Ktiles = []
                    for j_idx, jt in enumerate(grp):
                        E4 = work.tile([128, 512], BF16, tag="E", bufs=6)
                        nc.scalar.activation(
                            E4[:], gs[j_idx][:], ACTF.Exp, scale=sc[:, 4:5]
                        )
                        E3 = work.tile([128, 512], BF16, tag="Et", bufs=10)
                        nc.vector.tensor_tensor(E3[:], E4[:], E4[:], op=ALU.mult)
                        E2 = work.tile([128, 512], BF16, tag="Et", bufs=10)
                        nc.scalar.activation(E2[:], E3[:], ACTF.Square)
                        E1 = work.tile([128, 512], BF16, tag="Et", bufs=10)
                        nc.vector.tensor_tensor(E1[:], E2[:], E2[:], op=ALU.mult)
                        E0 = work.tile([128, 512], BF16, tag="Et", bufs=10)
                        nc.scalar.activation(E0[:], E1[:], ACTF.Square)
                        K1 = work.tile([128, 512], BF16, tag="Kt", bufs=8)
                        nc.vector.tensor_tensor(K1[:], E4[:], E3[:], op=ALU.add)
                        K2 = work.tile([128, 512], BF16, tag="Kt", bufs=8)
                        nc.vector.tensor_tensor(K2[:], K1[:], E2[:], op=ALU.add)
                        K3 = work.tile([128, 512], BF16, tag="Kt", bufs=8)
                        nc.vector.tensor_tensor(K3[:], K2[:], E1[:], op=ALU.add)
                        K4 = work.tile([128, 512], BF16, tag="Kf", bufs=8)
                        nc.vector.tensor_tensor(K4[:], K3[:], E0[:], op=ALU.add)
                        Ktiles.append(K4)
                    prev = (grp, i, Ktiles)

                # final pass's reduce + tail
                pgrp, pi, pK = prev
                _emit_reduce(
                    nc, work, mpsum, pgrp, pi, pK, vown_sb, racc_of_group[pgrp]
                )
                _emit_group_tail(
                    nc, work, pgrp, racc_of_group[pgrp], vt4_sb, loss_cols
                )

                nc.vector.tensor_reduce(lred[:], loss_cols[:], axis=AX.X, op=ALU.add)
                psum_f = mpsum.tile([1, 1], F32, tag="r", bufs=2)
                nc.tensor.matmul(
                    psum_f[:], lhsT=lred[:], rhs=onesf_col[:], start=True, stop=True
                )
                nc.vector.tensor_copy(out_sb[:], psum_f[:])
                nc.sync.dma_start(partial, out_sb[:])

    nc.compile()
    return nc


def _emit_reduce(nc, work, mpsum, grp, i, Ktiles, vown_sb, racc):
    """Single-shot V_blk^T @ K matmuls, accumulated over i in SBUF on DVE."""
    NC_ = CFG.ncls
    for j_idx, jt in enumerate(grp):
        rmm = mpsum.tile(
            [NC_, 512], mybir.dt.float32, tag="r", bufs=2, name=f"rmm{jt}_{i}"
        )
        nc.tensor.matmul(
            rmm[:], lhsT=vown_sb[:, i, :], rhs=Ktiles[j_idx][:],
            start=True, stop=True,
        )
        if i == 0:
            acc = work.tile(
                [NC_, 512], mybir.dt.float32, tag="racc", bufs=6,
                name=f"racc{jt}_{i}",
            )
            nc.vector.tensor_copy(acc[:], rmm[:])
        else:
            prev_acc = racc[j_idx]
            acc = work.tile(
                [NC_, 512], mybir.dt.float32, tag="racc", bufs=6,
                name=f"racc{jt}_{i}",
            )
            nc.vector.tensor_tensor(acc[:], prev_acc[:], rmm[:], op=ALU.add)
        racc[j_idx] = acc


def _emit_group_tail(nc, work, grp, racc, vt4_sb, loss_cols):
    """R (SBUF) x V^T -> loss_cols column, fused mult+reduce on DVE."""
    NC_ = CFG.ncls
    for j_idx, jt in enumerate(grp):
        scr = work.tile([NC_, 512], mybir.dt.float32, tag="scr", bufs=2)
        nc.vector.tensor_tensor(
            scr[:],
            racc[j_idx][:],
            vt4_sb[0:NC_, 512 * jt : 512 * (jt + 1)],
            op=ALU.mult,
        )
        nc.vector.tensor_reduce(
            loss_cols[0:NC_, jt : jt + 1], scr[:], axis=AX.X, op=ALU.add
        )


def host_prep(cfg: Cfg, source, target, s_label, t_label):
    """Slice/encode inputs into per-core in_maps (layout + dtype only)."""
    f8 = ml_dtypes.float8_e4m3
    bf16 = ml_dtypes.bfloat16
    X = np.concatenate(
        [np.asarray(source, np.float32), np.asarray(target, np.float32)], 0
    )
    N, D = X.shape
    bs = np.asarray(source).shape[0]
    lab = np.concatenate(
        [np.asarray(s_label).astype(np.int64), np.asarray(t_label).astype(np.int64)]
    )
    sign = np.ones(cfg.n, np.float32)
    sign[bs:] = -1.0
    V = np.zeros((cfg.n, cfg.ncls), np.float32)
    V[np.arange(cfg.n), lab] = sign
    Vb = V.astype(bf16)

    X8T = np.ascontiguousarray(X.astype(f8).T)          # [D, N]
    # [D, N] -> [nkk, 128, 2*N] with element (kk, p, t*N + j) = X8T[256kk+128t+p, j]
    xt8 = np.ascontiguousarray(
        X8T.reshape(cfg.nkk, 2, 128, N).transpose(0, 2, 1, 3).reshape(cfg.nkk, 128, 2 * N)
    )

    # V^T replicated at partition offsets {0, 32, 64, 96}
    vt4 = np.zeros((128, N), bf16)
    for m in range(4):
        vt4[32 * m : 32 * m + cfg.ncls, :] = Vb.T
    cones = np.ones((128, 1), bf16)
    conesf = np.ones((128, 1), np.float32)
    crowf = np.ones((1, 128), np.float32)
    conesN_h = np.ones((1, 4096), bf16)
    cst = np.zeros((1, 16), np.float32)
    for l in range(cfg.nl):
        cst[0, l] = 4.0 * (cfg.n - 1) / (2.0 ** l)

    in_maps = []
    for c in range(cfg.cores):
        r0, r1 = c * cfg.rpc, (c + 1) * cfg.rpc
        own = np.ascontiguousarray(X8T[:, r0:r1])        # [D, rpc]
        xto8 = np.ascontiguousarray(
            own.reshape(cfg.nkk, 2, 128, cfg.rpc)
            .transpose(0, 2, 1, 3)
            .reshape(cfg.nkk, 128, 2 * cfg.rpc)
        )
        vown = np.ascontiguousarray(
            Vb[r0:r1].reshape(cfg.ni, 128, cfg.ncls)
            .transpose(1, 0, 2)
            .reshape(128, cfg.ni * cfg.ncls)
        )
        in_maps.append(
            {
                "xt8": xt8,
                "xto8": xto8,
                "vown": vown,
                "vt4": vt4,
                "cones": cones,
                "conesf": conesf,
                "crowf": crowf,
                "cst": cst,
                "conesN": conesN_h,
            }
        )
    return in_maps


_NC_CACHE = {}


def _get_nc(cfg: Cfg):
    if cfg not in _NC_CACHE:
        _NC_CACHE[cfg] = _build(cfg)
    return _NC_CACHE[cfg]


def run(inputs: dict, cfg: Cfg = CFG, trace: bool = False):
    from concourse.bass_utils import run_bass_kernel_spmd

    nc = _get_nc(cfg)
    in_maps = host_prep(
        cfg,
        inputs["source"],
        inputs["target"],
        inputs["s_label"],
        inputs["t_label"],
    )
    res = run_bass_kernel_spmd(
        nc, in_maps, core_ids=list(range(cfg.cores)), trace=trace
    )
    bs = np.asarray(inputs["source"]).shape[0]
    total = sum(float(r["partial"][0, 0]) for r in res.results)
    loss = np.float32(total / float(bs) ** 2)
    return np.asarray(loss, dtype=np.float32), res


def kernel(**inputs) -> np.ndarray:
    out, _ = run(inputs)
    return out
